# revision 23
# baseline (speedup 1.0000x reference)
"""Trainium2 Bass kernel for nn_MultiHeadAttention_37039797961289.

MHA: B=1, S=4096, D=768, H=12, HD=64, fp32.

Sharding: the sequence dim is split into 8 slices of 512. Core c:
  - computes Q^T for its 512-row slice of x
  - redundantly computes the FULL K^T / V (collectives in this
    environment cost ~200us and idling the PE permanently throttles
    its clock, so 145us of replicated always-warm matmul is cheaper)
  - runs flash-style attention for its 512 queries over all 4096 keys
  - output-projects its 512 rows; host concatenates core outputs.

Layout choices:
  - Q^T/K^T are kept transposed [feat, seq] so the scores matmul
    (contraction over head_dim=64) needs no transposes.
  - scores^T tiles are [t=128, sq=512]; softmax runs along partitions:
    exp on ACT, the per-query denominator comes from a ones-column
    appended to V in the attention matmul (row 64 of the attn PSUM).
  - normalization: reciprocal (DVE) + rank-1 ones matmul broadcast.
  - matmuls use float32r (tf32-like, ~1e-4 rel err, 4x faster than fp32).
"""

import sys

sys.path.insert(0, "/opt/trn_rl_repo")

import numpy as np

import concourse.bass as bass
import concourse.mybir as mybir
import concourse.tile as tile
from concourse import bacc
from concourse.bass_utils import run_bass_kernel_spmd

FP32 = mybir.dt.float32
FP32R = mybir.dt.float32r
EXP = mybir.ActivationFunctionType.Exp
IDENT = mybir.ActivationFunctionType.Identity

N_CORES = 8
D = 768
H = 12
HD = 64
S = 4096
SQ = S // N_CORES  # 512 queries per core
KC = D // 128  # 6 contraction chunks of 128 over D
MP = 6  # 6 head-pair chunks of 128 rows in Q^T/K^T


def build_nc():
    nc = bacc.Bacc(None)

    xct = nc.dram_tensor("xct", [D, SQ], FP32, kind="ExternalInput")
    xt = nc.dram_tensor("xt", [D, S], FP32, kind="ExternalInput")
    wq = nc.dram_tensor("wq", [D, D], FP32, kind="ExternalInput")
    wk = nc.dram_tensor("wk", [D, D], FP32, kind="ExternalInput")
    wv = nc.dram_tensor("wv", [D, D], FP32, kind="ExternalInput")
    wo = nc.dram_tensor("wo", [HD, H, D], FP32, kind="ExternalInput")
    bq = nc.dram_tensor("bq", [128, MP], FP32, kind="ExternalInput")
    bk = nc.dram_tensor("bk", [128, MP], FP32, kind="ExternalInput")
    bv = nc.dram_tensor("bv", [1, D], FP32, kind="ExternalInput")
    bo = nc.dram_tensor("bo", [1, D], FP32, kind="ExternalInput")
    out = nc.dram_tensor("out", [SQ, D], FP32, kind="ExternalOutput")

    kt_full = nc.dram_tensor("kt_full", [D, S], FP32)
    v_full = nc.dram_tensor("v_full", [S, D], FP32)

    with tile.TileContext(nc) as tc:
        with tc.tile_pool(name="persist", bufs=1) as persist:

            # ---- resident tiles (DMA order: critical-path first) ----
            bk_sb = persist.tile([128, MP], FP32, name="bk_sb")
            nc.sync.dma_start(bk_sb[:], bk[:])
            bv_sb = persist.tile([1, D], FP32R, name="bv_sb")
            nc.sync.dma_start(bv_sb[:], bv[:].bitcast(FP32R))
            bq_sb = persist.tile([128, MP], FP32, name="bq_sb")
            nc.gpsimd.dma_start(bq_sb[:], bq[:])
            bo_sb = persist.tile([1, D], FP32R, name="bo_sb")
            nc.gpsimd.dma_start(bo_sb[:], bo[:].bitcast(FP32R))
            xct_sb = persist.tile([128, KC, SQ], FP32R, name="xct_sb")
            nc.gpsimd.dma_start(
                xct_sb[:], xct[:].rearrange("(o p) f -> p o f", p=128).bitcast(FP32R)
            )
            wo_sb = persist.tile([HD, H, D], FP32R, name="wo_sb")
            nc.gpsimd.dma_start(wo_sb[:], wo[:].bitcast(FP32R))
            ones32 = persist.tile([1, 128], FP32, name="ones32")
            nc.vector.memset(ones32[:], 1.0)
            ones_r = persist.tile([1, 128], FP32R, name="ones_r")
            nc.vector.tensor_copy(out=ones_r[:], in_=ones32[:])
            ones64 = ones_r[:, 0:HD]
            onescol32 = persist.tile([128, 1], FP32, name="onescol32")
            nc.vector.memset(onescol32[:], 1.0)
            onescol_r = persist.tile([128, 1], FP32R, name="onescol_r")
            nc.vector.tensor_copy(out=onescol_r[:], in_=onescol32[:])

            # Q^T m-chunks stay resident for all of phase C
            qt_sb = [
                persist.tile([128, SQ], FP32R, name=f"qt_{m}") for m in range(MP)
            ]
            # normalized attn^T per head, resident until phase D
            att_sb = [
                persist.tile([HD, SQ], FP32R, name=f"att_{h}") for h in range(H)
            ]


            # ---- phase A: projections ----
            with tc.tile_pool(name="wpool", bufs=1) as wpool, \
                 tc.tile_pool(name="xpool", bufs=2) as xpool, \
                 tc.tile_pool(name="evac", bufs=3) as evac, \
                 tc.tile_pool(name="psA", bufs=2, space="PSUM") as psA:
                wk_sb = wpool.tile([128, KC, D], FP32R, name="wk_sb")
                nc.sync.dma_start(
                    wk_sb[:], wk[:].rearrange("(o p) f -> p o f", p=128).bitcast(FP32R)
                )
                wv_sb = wpool.tile([128, KC, D], FP32R, name="wv_sb")
                nc.sync.dma_start(
                    wv_sb[:], wv[:].rearrange("(o p) f -> p o f", p=128).bitcast(FP32R)
                )
                wq_sb = wpool.tile([128, KC, D], FP32R, name="wq_sb")
                nc.gpsimd.dma_start(
                    wq_sb[:], wq[:].rearrange("(o p) f -> p o f", p=128).bitcast(FP32R)
                )

                # Full K^T and V projections, streaming x^T t-blocks.
                for j in range(N_CORES):
                    xtb = xpool.tile([128, KC, SQ], FP32R, name="xtb")
                    nc.sync.dma_start(
                        xtb[:],
                        xt[:, SQ * j : SQ * (j + 1)]
                        .rearrange("(o p) f -> p o f", p=128)
                        .bitcast(FP32R),
                    )
                    for m in range(MP):
                        ps = psA.tile([128, SQ], FP32, name="proj_ps")
                        for k in range(KC):
                            nc.tensor.matmul(
                                ps[:],
                                wk_sb[:, k, 128 * m : 128 * (m + 1)],
                                xtb[:, k, :],
                                start=(k == 0),
                                stop=(k == KC - 1),
                            )
                        kt_ev = evac.tile([128, SQ], FP32R, name="kt_ev")
                        nc.scalar.activation(
                            kt_ev[:], ps[:], IDENT, bias=bk_sb[:, m : m + 1]
                        )
                        nc.sync.dma_start(
                            kt_full[128 * m : 128 * (m + 1), SQ * j : SQ * (j + 1)],
                            kt_ev[:].bitcast(FP32),
                        )
                    for mt in range(SQ // 128):
                        for ns in range(2):
                            nsl = slice(384 * ns, 384 * (ns + 1))
                            ps = psA.tile([128, 384], FP32, name="v_ps")
                            for k in range(KC):
                                nc.tensor.matmul(
                                    ps[:],
                                    xtb[:, k, 128 * mt : 128 * (mt + 1)],
                                    wv_sb[:, k, nsl],
                                    start=(k == 0),
                                    stop=False,
                                )
                            nc.tensor.matmul(
                                ps[:], ones_r[:], bv_sb[:, nsl], start=False, stop=True
                            )
                            v_ev = evac.tile([128, 384], FP32, name="v_ev")
                            nc.vector.tensor_copy(out=v_ev[:], in_=ps[:])
                            nc.sync.dma_start(
                                v_full[SQ * j + 128 * mt : SQ * j + 128 * (mt + 1), nsl],
                                v_ev[:],
                            )

                for m in range(MP):
                    ps = psA.tile([128, SQ], FP32, name="proj_ps")
                    for k in range(KC):
                        nc.tensor.matmul(
                            ps[:],
                            wq_sb[:, k, 128 * m : 128 * (m + 1)],
                            xct_sb[:, k, :],
                            start=(k == 0),
                            stop=(k == KC - 1),
                        )
                    nc.scalar.activation(
                        qt_sb[m][:], ps[:], IDENT, bias=bq_sb[:, m : m + 1]
                    )

            # ---- phase C: attention ----
            with tc.tile_pool(name="kt_pool", bufs=3) as kt_pool, \
                 tc.tile_pool(name="v_pool", bufs=3) as v_pool, \
                 tc.tile_pool(name="exp_pool", bufs=2) as exp_pool, \
                 tc.tile_pool(name="ex_pool", bufs=3) as ex_pool, \
                 tc.tile_pool(name="sm_pool", bufs=2) as sm_pool, \
                 tc.tile_pool(name="pt_ps", bufs=2, space="PSUM") as pt_psp, \
                 tc.tile_pool(name="at_ps", bufs=2, space="PSUM") as at_psp:

                for m in range(MP):
                    h0, h1 = 2 * m, 2 * m + 1
                    at0 = at_psp.tile([HD + 1, SQ], FP32, name="at0")
                    at1 = at_psp.tile([HD + 1, SQ], FP32, name="at1")
                    prev = None
                    for r in range(N_CORES):
                        ktt = kt_pool.tile([128, SQ], FP32R, name="ktt")
                        nc.sync.dma_start(
                            ktt[:],
                            kt_full[
                                128 * m : 128 * (m + 1), SQ * r : SQ * (r + 1)
                            ].bitcast(FP32R),
                        )
                        # batched V loads: [512, 64] -> [128, 4, 64] (+ ones col)
                        v14 = []
                        for hh, h in enumerate((h0, h1)):
                            vt = v_pool.tile([128, 4, HD + 1], FP32R, name=f"v14_{hh}")
                            nc.sync.dma_start(
                                vt[:, :, 0:HD],
                                v_full[SQ * r : SQ * (r + 1), HD * h : HD * (h + 1)]
                                .rearrange("(o p) f -> p o f", p=128)
                                .bitcast(FP32R),
                            )
                            nc.gpsimd.tensor_copy(
                                out=vt[:, :, HD : HD + 1],
                                in_=onescol_r[:, 0:1, None].to_broadcast((128, 4, 1)),
                            )
                            v14.append(vt)

                        # scores for r (PE) + evac (DVE) + exp (ACT)
                        sc = [
                            exp_pool.tile([128, 4, 512], FP32, name=f"sc_{hh}")
                            for hh in range(2)
                        ]
                        for blk in range(2):
                            pts = [
                                pt_psp.tile([128, 1024], FP32, name="pt")
                                for _ in range(2)
                            ]
                            for jj in range(2):
                                j = 2 * blk + jj
                                for hh in range(2):
                                    prange = slice(64 * hh, 64 * (hh + 1))
                                    nc.tensor.matmul(
                                        pts[hh][:, 512 * jj : 512 * (jj + 1)],
                                        ktt[prange, 128 * j : 128 * (j + 1)],
                                        qt_sb[m][prange, :],
                                        start=True,
                                        stop=True,
                                    )
                            for hh in range(2):
                                nc.vector.tensor_copy(
                                    out=sc[hh][:, 2 * blk : 2 * blk + 2, :],
                                    in_=pts[hh][:],
                                )

                        exs = []
                        for hh in range(2):
                            ex = ex_pool.tile([128, 4, 512], FP32R, name="ex")
                            nc.scalar.activation(ex[:], sc[hh][:], EXP)
                            exs.append(ex)

                        # attention matmuls for the PREVIOUS r (exp already done)
                        if prev is not None:
                            pexs, pv14, pr = prev
                            for hh in range(2):
                                att_ps = at0 if hh == 0 else at1
                                for j in range(4):
                                    nc.tensor.matmul(
                                        att_ps[:],
                                        pv14[hh][:, j, :],
                                        pexs[hh][:, j, :],
                                        start=(pr == 0 and j == 0),
                                        stop=False,
                                    )
                        prev = (exs, v14, r)

                    # drain: attention for the last r
                    pexs, pv14, pr = prev
                    for hh in range(2):
                        att_ps = at0 if hh == 0 else at1
                        for j in range(4):
                            nc.tensor.matmul(
                                att_ps[:],
                                pv14[hh][:, j, :],
                                pexs[hh][:, j, :],
                                start=False,
                                stop=(j == 3),
                            )

                    # per-pair normalize: denom rows -> one [2,512] reciprocal
                    dnb = sm_pool.tile([HD + 1, SQ], FP32, name="dnb")
                    nc.vector.tensor_copy(out=dnb[HD : HD + 1, :], in_=at0[HD : HD + 1, :])
                    dnb2 = sm_pool.tile([HD + 1, SQ], FP32, name="dnb2")
                    nc.vector.tensor_copy(out=dnb2[HD : HD + 1, :], in_=at1[HD : HD + 1, :])
                    dn2 = sm_pool.tile([2, SQ], FP32, name="dn2")
                    nc.sync.dma_start(dn2[0:1, :], dnb[HD : HD + 1, :])
                    nc.sync.dma_start(dn2[1:2, :], dnb2[HD : HD + 1, :])
                    rec2 = sm_pool.tile([2, SQ], FP32, name="rec2")
                    nc.vector.reciprocal(rec2[:], dn2[:])
                    rec1b = sm_pool.tile([1, SQ], FP32, name="rec1b")
                    nc.sync.dma_start(rec1b[:], rec2[1:2, :])
                    for hh, h in enumerate((h0, h1)):
                        att_ps = at0 if hh == 0 else at1
                        bc_sb = sm_pool.tile([HD, SQ], FP32, name="bc_sb")
                        nc.gpsimd.partition_broadcast(
                            bc_sb[:], rec2[0:1, :] if hh == 0 else rec1b[:]
                        )
                        nc.vector.tensor_mul(
                            out=att_sb[h][:], in0=att_ps[0:HD, :], in1=bc_sb[:]
                        )

            # ---- phase D: output projection ----
            with tc.tile_pool(name="opool", bufs=3) as opool, \
                 tc.tile_pool(name="ops", bufs=2, space="PSUM") as ops:
                for i in range(SQ // 128):
                    for ns in range(2):
                        nsl = slice(384 * ns, 384 * (ns + 1))
                        ps = ops.tile([128, 384], FP32, name="o_ps")
                        for h in range(H):
                            nc.tensor.matmul(
                                ps[:],
                                att_sb[h][:, 128 * i : 128 * (i + 1)],
                                wo_sb[:, h, nsl],
                                start=(h == 0),
                                stop=False,
                            )
                        nc.tensor.matmul(
                            ps[:], ones_r[:], bo_sb[:, nsl], start=False, stop=True
                        )
                        o_ev = opool.tile([128, 384], FP32, name="o_ev")
                        nc.vector.tensor_copy(out=o_ev[:], in_=ps[:])
                        nc.sync.dma_start(out[128 * i : 128 * (i + 1), nsl], o_ev[:])

    nc.finalize()
    return nc


_NC_CACHE = None


def _get_nc():
    global _NC_CACHE
    if _NC_CACHE is None:
        _NC_CACHE = build_nc()
    return _NC_CACHE


def make_in_maps(hidden_states, Wq, Wk, Wv, bq, bk, bv, Wo, bo):
    x = np.asarray(hidden_states, dtype=np.float32)[0]  # [S, D]
    scale = 1.0 / np.sqrt(np.float32(HD))

    xT = np.ascontiguousarray(x.T)  # [D, S]
    wq_all = np.ascontiguousarray(
        (np.asarray(Wq) * scale).transpose(1, 0, 2).reshape(D, D).astype(np.float32)
    )
    wk_all = np.ascontiguousarray(
        np.asarray(Wk).transpose(1, 0, 2).reshape(D, D).astype(np.float32)
    )
    wv_all = np.ascontiguousarray(
        np.asarray(Wv).transpose(1, 0, 2).reshape(D, D).astype(np.float32)
    )
    wo_r = np.ascontiguousarray(
        np.asarray(Wo, dtype=np.float32).reshape(H, HD, D).transpose(1, 0, 2)
    )  # [HD, H, D]
    bq_r = np.ascontiguousarray(
        (np.asarray(bq) * scale).reshape(D).reshape(MP, 128).T.astype(np.float32)
    )  # [128, MP]
    bk_r = np.ascontiguousarray(
        np.asarray(bk, dtype=np.float32).reshape(D).reshape(MP, 128).T
    )
    bv_r = np.asarray(bv, dtype=np.float32).reshape(1, D)
    bo_r = np.asarray(bo, dtype=np.float32).reshape(1, D)

    in_maps = []
    for c in range(N_CORES):
        in_maps.append(
            {
                "xct": np.ascontiguousarray(xT[:, SQ * c : SQ * (c + 1)]),
                "xt": xT,
                "wq": wq_all,
                "wk": wk_all,
                "wv": wv_all,
                "wo": wo_r,
                "bq": bq_r,
                "bk": bk_r,
                "bv": bv_r,
                "bo": bo_r,
            }
        )
    return in_maps


def kernel(hidden_states, Wq, Wk, Wv, bq, bk, bv, Wo, bo):
    in_maps = make_in_maps(hidden_states, Wq, Wk, Wv, bq, bk, bv, Wo, bo)
    nc = _get_nc()
    res = run_bass_kernel_spmd(nc, in_maps, list(range(N_CORES)))
    outs = [res.results[c]["out"] for c in range(N_CORES)]
    return np.concatenate(outs, axis=0)[None, :, :].astype(np.float32)
